# revision 27
# baseline (speedup 1.0000x reference)
"""Multi-head attention (B=2, T=2048, D=1024, H=16) on 8 TRN2 NeuronCores.

Sharding: 2D (batch x head-group). Core c handles batch b = c // 4 and head
group hg = c % 4 (4 heads = 256 channels of the projected dim). Each core:
  1. Projects its batch's q/k/v against its 256-row weight slices -> QT/KT
     in [j, t] layout and V in [t, j] layout (bf16, fp32 PSUM accumulation).
     V is stored augmented with a ones column per head: [V_h | 1].
     Order Q, V, K so attention never stalls waiting for V.
  2. Per head pair, per 512-wide q tile: S.T = K_h @ Q_h.T (transposed
     scores), U = exp(S.T * scale) (no max subtraction: |S*scale| <= ~16,
     exp fits fp32 easily), then [O.T ; denom] += [V_h | 1].T @ U -- the
     softmax denominator rides the PV matmul for free as output row 64.
     The PV matmuls trail the score/exp stage by one k tile so the PE
     never waits on ScalarE (keeps the HAM clock at 2.4 GHz).
  3. Raw [O.T ; denom] is staged to SBUF; per-block reciprocals run on
     idle DVE cycles; normalization + the output projection for q tile
     qt-1 are woven into the middle of qt's blocks as PE filler.
  4. out_partial.T = woT_chunk.T @ O_norm.T  -> [1024, 2048] fp32.
Host sums the 4 head-group partials per batch, transposes, adds bo.

PSUM discipline: exactly one accumulation group per PSUM bank (hardware
start=True clears has_written bits bank-wide). Engine ops only start at
partition offsets {0, 32, 64, 96}; partition shifts (head m=1 belongs at
rows 64-127 of the stage-E operand but results sit at rows 0-64) use
small SBUF->SBUF DMAs.

All shapes are hardcoded for this problem. kernel() takes the full inputs
and returns the full [2, 2048, 1024] fp32 output.
"""

import numpy as np
import ml_dtypes

import concourse.bass as bass
import concourse.bacc as bacc
import concourse.mybir as mybir
import concourse.tile as tile
from concourse.bass_utils import run_bass_kernel_spmd

B, T, D, H, Hd = 2, 2048, 1024, 16, 64
HPC = 4          # heads per core
W = HPC * Hd     # 256 projected channels per core
SCALE = Hd ** -0.5
N_CORES = 8

BF16 = mybir.dt.bfloat16
F32 = mybir.dt.float32
bf16 = ml_dtypes.bfloat16


def build_nc():
    nc = bacc.Bacc("TRN2", target_bir_lowering=False, debug=False)

    xq = nc.dram_tensor("xq", [D, T], BF16, kind="ExternalInput").ap()
    xk = nc.dram_tensor("xk", [D, T], BF16, kind="ExternalInput").ap()
    xv = nc.dram_tensor("xv", [D, T], BF16, kind="ExternalInput").ap()
    # weights host-preswizzled to [128, chunk, cols] DMA-contiguous layout
    wq = nc.dram_tensor("wq", [128, 8 * W], BF16, kind="ExternalInput").ap()
    wk = nc.dram_tensor("wk", [128, 8 * W], BF16, kind="ExternalInput").ap()
    wv = nc.dram_tensor("wv", [128, 8 * W], BF16, kind="ExternalInput").ap()
    wo = nc.dram_tensor("wo", [128, 2 * D], BF16, kind="ExternalInput").ap()
    bq = nc.dram_tensor("bq", [1, W], BF16, kind="ExternalInput").ap()
    bk = nc.dram_tensor("bk", [1, W], BF16, kind="ExternalInput").ap()
    bv = nc.dram_tensor("bv", [1, W], BF16, kind="ExternalInput").ap()
    ident = nc.dram_tensor("ident", [128, 128], BF16, kind="ExternalInput").ap()
    out = nc.dram_tensor("out", [D, T], F32, kind="ExternalOutput").ap()

    Exp = mybir.ActivationFunctionType.Exp

    with tile.TileContext(nc) as tc:
        with (
            tc.tile_pool(name="persist", bufs=1) as persist,
            tc.tile_pool(name="xpool", bufs=8) as xpool,
            tc.tile_pool(name="upool", bufs=8) as upool,
            tc.tile_pool(name="rpool", bufs=2) as rpool,
            tc.tile_pool(name="opool", bufs=4) as opool,
        ):
            # ---- constants ----
            ones_row = persist.tile([1, 512], BF16, tag="ones_row")
            nc.vector.memset(ones_row, 1.0)
            ident_sb = persist.tile([128, 128], BF16, tag="ident")
            nc.sync.dma_start(out=ident_sb, in_=ident)
            # K=1 broadcast matmul stationary: ones row at partition 64
            bcast1 = persist.tile([65, 64], BF16, tag="bcast1")
            nc.vector.memset(bcast1[64:65, :], 1.0)

            # ---- weights / biases ----
            # K's weights first -- the first matmuls wait only on these
            wk_sb = persist.tile([128, 8, W], BF16, tag="wk")
            nc.sync.dma_start(out=wk_sb, in_=wk.rearrange("p (c j) -> p c j", j=W))
            bk_sb = persist.tile([1, W], BF16, tag="bk")
            nc.sync.dma_start(out=bk_sb, in_=bk)
            wq_sb = persist.tile([128, 8, W], BF16, tag="wq")
            nc.sync.dma_start(out=wq_sb, in_=wq.rearrange("p (c j) -> p c j", j=W))
            bq_sb = persist.tile([1, W], BF16, tag="bq")
            nc.sync.dma_start(out=bq_sb, in_=bq)
            wv_sb = persist.tile([128, 8, W], BF16, tag="wv")
            nc.sync.dma_start(out=wv_sb, in_=wv.rearrange("p (c j) -> p c j", j=W))
            bv_sb = persist.tile([1, W], BF16, tag="bv")
            nc.sync.dma_start(out=bv_sb, in_=bv)
            wo_sb = persist.tile([128, 2, D], BF16, tag="wo")
            nc.sync.dma_start(out=wo_sb, in_=wo.rearrange("p (c e) -> p c e", e=D))

            # ---- persistent activations ----
            qt_sb = persist.tile([128, 2, T], BF16, tag="qt")   # QT [j, t]
            kt_sb = persist.tile([128, 2, T], BF16, tag="kt")   # KT [j, t]
            # V augmented with ones column per head: [k, kt, h, 0:64]=V, [..64]=1
            vaug_sb = persist.tile([128, 16, HPC, Hd + 1], BF16, tag="vaug")
            nc.vector.memset(vaug_sb[:, :, :, 64:65], 1.0)
            otn_sb = persist.tile([128, 2, T], BF16, tag="otn")  # normalized O.T
            vt_sb = persist.tile([128, 2, T], BF16, tag="vt")    # V.T [j, t]
            # raw [O.T ; denom] per block b2 = (pr*4+qt)*2 + m
            oraw_sb = persist.tile([65, 16, 512], F32, tag="oraw")

            # ================= Phase A: projections =================
            with tc.tile_pool(name="psA", bufs=8, space="PSUM") as psA:
                def qk_proj(x_dram, w_sb, b_sb, dst):
                    ps = [psA.tile([128, 512], F32, tag="proj", name=f"proj{i}")
                          for i in range(8)]
                    for c in range(8):
                        xc = xpool.tile([128, T], BF16, tag="x", name="xc")
                        nc.gpsimd.dma_start(out=xc,
                                            in_=x_dram[c * 128:(c + 1) * 128, :])
                        for jt in range(2):
                            for tt in range(4):
                                nc.tensor.matmul(
                                    ps[jt * 4 + tt],
                                    lhsT=w_sb[:, c, jt * 128:(jt + 1) * 128],
                                    rhs=xc[:, tt * 512:(tt + 1) * 512],
                                    start=(c == 0), stop=False,
                                )
                    for jt in range(2):
                        for tt in range(4):
                            p = ps[jt * 4 + tt]
                            nc.tensor.matmul(
                                p, lhsT=b_sb[:, jt * 128:(jt + 1) * 128],
                                rhs=ones_row, start=False, stop=True,
                            )
                            nc.vector.tensor_copy(
                                dst[:, jt, tt * 512:(tt + 1) * 512], p)

                def v_proj():
                    # V.T in [j, t] layout like Q/K (stationary weights, one
                    # pass over xv), then PE transposes into vaug's [t, j].
                    qk_proj(xv, wv_sb, bv_sb, vt_sb)
                    for jt in range(2):
                        for tt in range(16):
                            tp = psA.tile([128, 128], BF16, tag="proj",
                                          name="tp")
                            nc.tensor.transpose(
                                tp, vt_sb[:, jt, tt * 128:(tt + 1) * 128],
                                ident_sb)
                            nc.vector.tensor_copy(
                                vaug_sb[:, tt, 2 * jt:2 * jt + 2, 0:64],
                                tp.rearrange("t (h d) -> t h d", h=2))

                # K then Q so the attention score matmuls + exp can begin
                # while the V projection still runs (V only gates the PV
                # matmuls, which trail by a k tile anyway).
                qk_proj(xk, wk_sb, bk_sb, kt_sb)
                qk_proj(xq, wq_sb, bq_sb, qt_sb)
                v_proj()

            # ====== Phase B/D + fused normalization/output projection ======
            with tc.tile_pool(name="psB", bufs=1, space="PSUM") as psB:
                recips = {}

                def attn_block(pr, qt, fillers=()):
                    fillers = dict(fillers)
                    qsl = slice(qt * 512, (qt + 1) * 512)
                    o_psA = psB.tile([65, 512], F32, tag="oA", bufs=1,
                                     name="o_psA")
                    o_psB = psB.tile([65, 512], F32, tag="oB", bufs=1,
                                     name="o_psB")
                    us = []
                    for kt in range(17):
                        if kt < 16:
                            s_big = psB.tile([128, 2, 512], F32, tag="s",
                                             bufs=2, name="s_big")
                            for m in range(2):
                                po = 64 * m
                                nc.tensor.matmul(
                                    s_big[:, m, :],
                                    lhsT=kt_sb[po:po + 64, pr,
                                               kt * 128:(kt + 1) * 128],
                                    rhs=qt_sb[po:po + 64, pr, qsl],
                                    start=True, stop=True,
                                )
                            u_big = upool.tile([128, 2, 512], BF16, tag="u",
                                               name="u_big")
                            nc.scalar.activation(u_big, s_big, Exp, scale=SCALE)
                            us.append(u_big)
                        if kt >= 1:
                            for m, o_ps in ((0, o_psA), (1, o_psB)):
                                h = 2 * pr + m
                                nc.tensor.matmul(
                                    o_ps,
                                    lhsT=vaug_sb[:, kt - 1, h, :],
                                    rhs=us[kt - 1][:, m, :],
                                    start=(kt == 1), stop=(kt == 16),
                                )
                        # weave prior-tile normalization / projection work
                        # into the loop so ScalarE never starves at block
                        # boundaries
                        if kt in fillers:
                            fillers.pop(kt)()
                    for fn in fillers.values():
                        fn()
                    # fast reciprocal of each denominator row straight from
                    # PSUM (unblocks the woven rb fillers early), bf16 cast
                    # for the broadcast matmul, then stage raw results.
                    # stage raw results; per-head reciprocal of the
                    # denominator row (partition 64), bf16 cast for the
                    # broadcast matmul. Runs on DVE slack during the next
                    # block; the woven rb fillers are scheduled late enough.
                    for m, o_ps in ((0, o_psA), (1, o_psB)):
                        b2 = (pr * 4 + qt) * 2 + m
                        rt = rpool.tile([65, 512], F32, tag="rt", bufs=8,
                                        name="rt")
                        nc.vector.reciprocal(rt[64:65, :], o_ps[64:65, :])
                        rtb = rpool.tile([65, 512], BF16, tag="rtb", bufs=8,
                                         name="rtb")
                        with nc.allow_low_precision(
                                reason="1/denom bf16; ample for softmax"):
                            nc.vector.tensor_copy(rtb[64:65, :], rt[64:65, :])
                        recips[b2] = rtb
                    for m, o_ps in ((0, o_psA), (1, o_psB)):
                        b2 = (pr * 4 + qt) * 2 + m
                        nc.vector.tensor_copy(oraw_sb[:, b2, :], o_ps)

                def norm_pieces(qt):
                    # normalize O.T for q tile qt: 4 filler closures
                    qsl = slice(qt * 512, (qt + 1) * 512)

                    def piece(pr, m):
                        def run():
                            b2 = (pr * 4 + qt) * 2 + m
                            rb_ps = psB.tile([64, 512], F32, tag="rb", bufs=1,
                                             name="rb_ps")
                            nc.tensor.matmul(
                                rb_ps, lhsT=bcast1[64:65, :],
                                rhs=recips[b2][64:65, :],
                                start=True, stop=True)
                            rb_sb = rpool.tile([64, 512], F32, tag="rbs",
                                               name="rb_sb")
                            nc.vector.tensor_copy(rb_sb, rb_ps)
                            if m == 0:
                                nc.vector.tensor_mul(
                                    otn_sb[0:64, pr, qsl],
                                    oraw_sb[0:64, b2, :], rb_sb)
                            else:
                                otnB = rpool.tile([64, 512], BF16, tag="otnB",
                                                  name="otnB")
                                nc.vector.tensor_mul(
                                    otnB, oraw_sb[0:64, b2, :], rb_sb)
                                nc.sync.dma_start(
                                    out=otn_sb[64:128, pr, qsl], in_=otnB)
                        return run
                    # later slots: (pr=1) reciprocals are issued at the
                    # immediately preceding block boundary and need ~7us
                    return [(6, piece(0, 0)), (9, piece(0, 1)),
                            (12, piece(1, 0)), (15, piece(1, 1))]

                def proj_pieces(qt):
                    # output projection for q tile qt: 8 filler closures
                    qsl = slice(qt * 512, (qt + 1) * 512)

                    def piece(et):
                        def run():
                            e_ps = psB.tile([128, 512], F32, tag="e", bufs=1,
                                            name="e_ps")
                            for jc in range(2):
                                nc.tensor.matmul(
                                    e_ps,
                                    lhsT=wo_sb[:, jc, et * 128:(et + 1) * 128],
                                    rhs=otn_sb[:, jc, qsl],
                                    start=(jc == 0), stop=(jc == 1),
                                )
                            stg = opool.tile([128, 512], F32, tag="ostg",
                                             name="stg")
                            nc.vector.tensor_copy(stg, e_ps)
                            nc.sync.dma_start(
                                out=out[et * 128:(et + 1) * 128, qsl], in_=stg)
                        return run
                    return [(2 * et + 2, piece(et)) for et in range(8)]

                for qt in range(4):
                    attn_block(0, qt,
                               fillers=norm_pieces(qt - 1) if qt >= 1 else ())
                    attn_block(1, qt,
                               fillers=proj_pieces(qt - 1) if qt >= 1 else ())
                for _, f in norm_pieces(3):
                    f()
                for _, f in proj_pieces(3):
                    f()

    nc.finalize()
    return nc


_NC_CACHE = None


def _get_nc():
    global _NC_CACHE
    if _NC_CACHE is None:
        _NC_CACHE = build_nc()
    return _NC_CACHE


def _swz(wT):
    """[C*128, cols] -> DMA-contiguous [128, C*cols] (partition-major)."""
    C = wT.shape[0] // 128
    return np.ascontiguousarray(
        wT.reshape(C, 128, -1).swapaxes(0, 1).reshape(128, -1)).astype(bf16)


def make_in_maps(query, key, value, wq, bq, wk, bk, wv, bv, wo, bo):
    in_maps = []
    for c in range(N_CORES):
        b, hg = divmod(c, HPC)
        sl = slice(hg * W, (hg + 1) * W)
        in_maps.append({
            "xq": np.ascontiguousarray(np.asarray(query[b]).T).astype(bf16),
            "xk": np.ascontiguousarray(np.asarray(key[b]).T).astype(bf16),
            "xv": np.ascontiguousarray(np.asarray(value[b]).T).astype(bf16),
            "wq": _swz(np.asarray(wq)[sl].T),
            "wk": _swz(np.asarray(wk)[sl].T),
            "wv": _swz(np.asarray(wv)[sl].T),
            "wo": _swz(np.asarray(wo)[:, sl].T),
            "bq": np.asarray(bq)[sl].reshape(1, W).astype(bf16),
            "bk": np.asarray(bk)[sl].reshape(1, W).astype(bf16),
            "bv": np.asarray(bv)[sl].reshape(1, W).astype(bf16),
            "ident": np.eye(128, dtype=np.float32).astype(bf16),
        })
    return in_maps


def combine_outputs(outs, bo):
    full = np.zeros((B, T, D), np.float32)
    for c in range(N_CORES):
        b = c // HPC
        full[b] += outs[c].T
    full += np.asarray(bo, np.float32)[None, None, :]
    return full


def kernel(query, key, value, wq, bq, wk, bk, wv, bv, wo, bo):
    nc = _get_nc()
    in_maps = make_in_maps(query, key, value, wq, bq, wk, bk, wv, bv, wo, bo)
    res = run_bass_kernel_spmd(nc, in_maps, list(range(N_CORES)))
    outs = [np.asarray(res.results[c]["out"]) for c in range(N_CORES)]
    return combine_outputs(outs, bo)


# revision 28
# speedup vs baseline: 1.0403x; 1.0403x over previous
"""Multi-head attention (B=2, T=2048, D=1024, H=16) on 8 TRN2 NeuronCores.

Sharding: 2D (batch x head-group). Core c handles batch b = c // 4 and head
group hg = c % 4 (4 heads = 256 channels of the projected dim). Each core:
  1. Projects its batch's q/k/v against its 256-row weight slices -> QT/KT
     in [j, t] layout and V in [t, j] layout (bf16, fp32 PSUM accumulation).
     V is stored augmented with a ones column per head: [V_h | 1].
     Order Q, V, K so attention never stalls waiting for V.
  2. Per head pair, per 512-wide q tile: S.T = K_h @ Q_h.T (transposed
     scores), U = exp(S.T * scale) (no max subtraction: |S*scale| <= ~16,
     exp fits fp32 easily), then [O.T ; denom] += [V_h | 1].T @ U -- the
     softmax denominator rides the PV matmul for free as output row 64.
     The PV matmuls trail the score/exp stage by one k tile so the PE
     never waits on ScalarE (keeps the HAM clock at 2.4 GHz).
  3. Raw [O.T ; denom] is staged to SBUF; per-block reciprocals run on
     idle DVE cycles; normalization + the output projection for q tile
     qt-1 are woven into the middle of qt's blocks as PE filler.
  4. out_partial.T = woT_chunk.T @ O_norm.T  -> [1024, 2048] fp32.
Host sums the 4 head-group partials per batch, transposes, adds bo.

PSUM discipline: exactly one accumulation group per PSUM bank (hardware
start=True clears has_written bits bank-wide). Engine ops only start at
partition offsets {0, 32, 64, 96}; partition shifts (head m=1 belongs at
rows 64-127 of the stage-E operand but results sit at rows 0-64) use
small SBUF->SBUF DMAs.

All shapes are hardcoded for this problem. kernel() takes the full inputs
and returns the full [2, 2048, 1024] fp32 output.
"""

import numpy as np
import ml_dtypes

import concourse.bass as bass
import concourse.bacc as bacc
import concourse.mybir as mybir
import concourse.tile as tile
from concourse.bass_utils import run_bass_kernel_spmd

B, T, D, H, Hd = 2, 2048, 1024, 16, 64
HPC = 4          # heads per core
W = HPC * Hd     # 256 projected channels per core
SCALE = Hd ** -0.5
N_CORES = 8

BF16 = mybir.dt.bfloat16
F32 = mybir.dt.float32
bf16 = ml_dtypes.bfloat16


def build_nc():
    nc = bacc.Bacc("TRN2", target_bir_lowering=False, debug=False)

    xq = nc.dram_tensor("xq", [D, T], BF16, kind="ExternalInput").ap()
    xk = nc.dram_tensor("xk", [D, T], BF16, kind="ExternalInput").ap()
    xv = nc.dram_tensor("xv", [D, T], BF16, kind="ExternalInput").ap()
    # weights host-preswizzled to [128, chunk, cols] DMA-contiguous layout
    wq = nc.dram_tensor("wq", [128, 8 * W], BF16, kind="ExternalInput").ap()
    wk = nc.dram_tensor("wk", [128, 8 * W], BF16, kind="ExternalInput").ap()
    wv = nc.dram_tensor("wv", [128, 8 * W], BF16, kind="ExternalInput").ap()
    wo = nc.dram_tensor("wo", [128, 2 * D], BF16, kind="ExternalInput").ap()
    bq = nc.dram_tensor("bq", [1, W], BF16, kind="ExternalInput").ap()
    bk = nc.dram_tensor("bk", [1, W], BF16, kind="ExternalInput").ap()
    bv = nc.dram_tensor("bv", [1, W], BF16, kind="ExternalInput").ap()
    ident = nc.dram_tensor("ident", [128, 128], BF16, kind="ExternalInput").ap()
    out = nc.dram_tensor("out", [D, T], F32, kind="ExternalOutput").ap()

    Exp = mybir.ActivationFunctionType.Exp

    with tile.TileContext(nc) as tc:
        with (
            tc.tile_pool(name="persist", bufs=1) as persist,
            tc.tile_pool(name="xpool", bufs=8) as xpool,
            tc.tile_pool(name="upool", bufs=8) as upool,
            tc.tile_pool(name="rpool", bufs=2) as rpool,
            tc.tile_pool(name="opool", bufs=4) as opool,
        ):
            # ---- constants ----
            ones_row = persist.tile([1, 512], BF16, tag="ones_row")
            nc.vector.memset(ones_row, 1.0)
            ident_sb = persist.tile([128, 128], BF16, tag="ident")
            nc.sync.dma_start(out=ident_sb, in_=ident)
            # K=1 broadcast matmul stationary: ones row at partition 64
            bcast1 = persist.tile([65, 64], BF16, tag="bcast1")
            nc.vector.memset(bcast1[64:65, :], 1.0)

            # ---- weights / biases ----
            # K's weights first -- the first matmuls wait only on these
            wk_sb = persist.tile([128, 8, W], BF16, tag="wk")
            nc.sync.dma_start(out=wk_sb, in_=wk.rearrange("p (c j) -> p c j", j=W))
            bk_sb = persist.tile([1, W], BF16, tag="bk")
            nc.sync.dma_start(out=bk_sb, in_=bk)
            wq_sb = persist.tile([128, 8, W], BF16, tag="wq")
            nc.sync.dma_start(out=wq_sb, in_=wq.rearrange("p (c j) -> p c j", j=W))
            bq_sb = persist.tile([1, W], BF16, tag="bq")
            nc.sync.dma_start(out=bq_sb, in_=bq)
            wv_sb = persist.tile([128, 8, W], BF16, tag="wv")
            nc.sync.dma_start(out=wv_sb, in_=wv.rearrange("p (c j) -> p c j", j=W))
            bv_sb = persist.tile([1, W], BF16, tag="bv")
            nc.sync.dma_start(out=bv_sb, in_=bv)
            wo_sb = persist.tile([128, 2, D], BF16, tag="wo")
            nc.sync.dma_start(out=wo_sb, in_=wo.rearrange("p (c e) -> p c e", e=D))

            # ---- persistent activations ----
            qt_sb = persist.tile([128, 2, T], BF16, tag="qt")   # QT [j, t]
            kt_sb = persist.tile([128, 2, T], BF16, tag="kt")   # KT [j, t]
            # V augmented with ones column per head: [k, kt, h, 0:64]=V, [..64]=1
            vaug_sb = persist.tile([128, 16, HPC, Hd + 1], BF16, tag="vaug")
            nc.vector.memset(vaug_sb[:, :, :, 64:65], 1.0)
            otn_sb = persist.tile([128, 2, T], BF16, tag="otn")  # normalized O.T
            vt_sb = persist.tile([128, 2, T], BF16, tag="vt")    # V.T [j, t]
            # raw [O.T ; denom] per block b2 = (pr*4+qt)*2 + m
            oraw_sb = persist.tile([65, 16, 512], F32, tag="oraw")

            # ================= Phase A: projections =================
            with tc.tile_pool(name="psA", bufs=8, space="PSUM") as psA:
                def qk_proj(x_dram, w_sb, b_sb, dst):
                    ps = [psA.tile([128, 512], F32, tag="proj", name=f"proj{i}")
                          for i in range(8)]
                    for c in range(8):
                        xc = xpool.tile([128, T], BF16, tag="x", name="xc")
                        nc.gpsimd.dma_start(out=xc,
                                            in_=x_dram[c * 128:(c + 1) * 128, :])
                        for jt in range(2):
                            for tt in range(4):
                                nc.tensor.matmul(
                                    ps[jt * 4 + tt],
                                    lhsT=w_sb[:, c, jt * 128:(jt + 1) * 128],
                                    rhs=xc[:, tt * 512:(tt + 1) * 512],
                                    start=(c == 0), stop=False,
                                )
                    for jt in range(2):
                        for tt in range(4):
                            p = ps[jt * 4 + tt]
                            nc.tensor.matmul(
                                p, lhsT=b_sb[:, jt * 128:(jt + 1) * 128],
                                rhs=ones_row, start=False, stop=True,
                            )
                            nc.vector.tensor_copy(
                                dst[:, jt, tt * 512:(tt + 1) * 512], p)

                def v_proj():
                    # V.T in [j, t] layout like Q/K (stationary weights, one
                    # pass over xv), then PE transposes into vaug's [t, j].
                    qk_proj(xv, wv_sb, bv_sb, vt_sb)
                    for jt in range(2):
                        for tt in range(16):
                            tp = psA.tile([128, 128], BF16, tag="proj",
                                          name="tp")
                            nc.tensor.transpose(
                                tp, vt_sb[:, jt, tt * 128:(tt + 1) * 128],
                                ident_sb)
                            nc.vector.tensor_copy(
                                vaug_sb[:, tt, 2 * jt:2 * jt + 2, 0:64],
                                tp.rearrange("t (h d) -> t h d", h=2))

                # K then Q so the attention score matmuls + exp can begin
                # while the V projection still runs (V only gates the PV
                # matmuls, which trail by a k tile anyway).
                qk_proj(xk, wk_sb, bk_sb, kt_sb)
                qk_proj(xq, wq_sb, bq_sb, qt_sb)
                v_proj()

            # ====== Phase B/D + fused normalization/output projection ======
            with tc.tile_pool(name="psB", bufs=1, space="PSUM") as psB:
                recips = {}

                def attn_block(pr, qt, fillers=()):
                    fillers = dict(fillers)
                    qsl = slice(qt * 512, (qt + 1) * 512)
                    o_psA = psB.tile([65, 512], F32, tag="oA", bufs=1,
                                     name="o_psA")
                    o_psB = psB.tile([65, 512], F32, tag="oB", bufs=1,
                                     name="o_psB")
                    us = []
                    for kt in range(17):
                        if kt < 16:
                            s_big = psB.tile([128, 2, 512], F32, tag="s",
                                             bufs=2, name="s_big")
                            for m in range(2):
                                po = 64 * m
                                nc.tensor.matmul(
                                    s_big[:, m, :],
                                    lhsT=kt_sb[po:po + 64, pr,
                                               kt * 128:(kt + 1) * 128],
                                    rhs=qt_sb[po:po + 64, pr, qsl],
                                    start=True, stop=True,
                                )
                            u_big = upool.tile([128, 2, 512], BF16, tag="u",
                                               name="u_big")
                            nc.scalar.activation(u_big, s_big, Exp, scale=SCALE)
                            us.append(u_big)
                        if kt >= 1:
                            for m, o_ps in ((0, o_psA), (1, o_psB)):
                                h = 2 * pr + m
                                nc.tensor.matmul(
                                    o_ps,
                                    lhsT=vaug_sb[:, kt - 1, h, :],
                                    rhs=us[kt - 1][:, m, :],
                                    start=(kt == 1), stop=(kt == 16),
                                )
                        # weave prior-tile normalization / projection work
                        # into the loop so ScalarE never starves at block
                        # boundaries
                        if kt in fillers:
                            fillers.pop(kt)()
                    for fn in fillers.values():
                        fn()
                    # fast reciprocal of each denominator row straight from
                    # PSUM (unblocks the woven rb fillers early), bf16 cast
                    # for the broadcast matmul, then stage raw results.
                    # stage raw results; per-head reciprocal of the
                    # denominator row (partition 64), bf16 cast for the
                    # broadcast matmul. Runs on DVE slack during the next
                    # block; the woven rb fillers are scheduled late enough.
                    for m, o_ps in ((0, o_psA), (1, o_psB)):
                        b2 = (pr * 4 + qt) * 2 + m
                        nc.vector.tensor_copy(oraw_sb[:, b2, :], o_ps)
                        rt = rpool.tile([65, 512], F32, tag="rt", bufs=8,
                                        name="rt")
                        nc.vector.reciprocal(rt[64:65, :],
                                             oraw_sb[64:65, b2, :])
                        rtb = rpool.tile([65, 512], BF16, tag="rtb", bufs=8,
                                         name="rtb")
                        with nc.allow_low_precision(
                                reason="1/denom bf16; ample for softmax"):
                            nc.vector.tensor_copy(rtb[64:65, :], rt[64:65, :])
                        recips[b2] = rtb

                def norm_pieces(qt):
                    # normalize O.T for q tile qt: 4 filler closures
                    qsl = slice(qt * 512, (qt + 1) * 512)

                    def piece(pr, m):
                        def run():
                            b2 = (pr * 4 + qt) * 2 + m
                            rb_ps = psB.tile([64, 512], F32, tag="rb", bufs=1,
                                             name="rb_ps")
                            nc.tensor.matmul(
                                rb_ps, lhsT=bcast1[64:65, :],
                                rhs=recips[b2][64:65, :],
                                start=True, stop=True)
                            rb_sb = rpool.tile([64, 512], F32, tag="rbs",
                                               name="rb_sb")
                            nc.vector.tensor_copy(rb_sb, rb_ps)
                            if m == 0:
                                nc.vector.tensor_mul(
                                    otn_sb[0:64, pr, qsl],
                                    oraw_sb[0:64, b2, :], rb_sb)
                            else:
                                otnB = rpool.tile([64, 512], BF16, tag="otnB",
                                                  name="otnB")
                                nc.vector.tensor_mul(
                                    otnB, oraw_sb[0:64, b2, :], rb_sb)
                                nc.sync.dma_start(
                                    out=otn_sb[64:128, pr, qsl], in_=otnB)
                        return run
                    # later slots: (pr=1) reciprocals are issued at the
                    # immediately preceding block boundary and need ~7us
                    return [(6, piece(0, 0)), (9, piece(0, 1)),
                            (12, piece(1, 0)), (15, piece(1, 1))]

                def proj_pieces(qt):
                    # output projection for q tile qt: 8 filler closures
                    qsl = slice(qt * 512, (qt + 1) * 512)

                    def piece(et):
                        def run():
                            e_ps = psB.tile([128, 512], F32, tag="e", bufs=1,
                                            name="e_ps")
                            for jc in range(2):
                                nc.tensor.matmul(
                                    e_ps,
                                    lhsT=wo_sb[:, jc, et * 128:(et + 1) * 128],
                                    rhs=otn_sb[:, jc, qsl],
                                    start=(jc == 0), stop=(jc == 1),
                                )
                            stg = opool.tile([128, 512], F32, tag="ostg",
                                             name="stg")
                            nc.vector.tensor_copy(stg, e_ps)
                            nc.sync.dma_start(
                                out=out[et * 128:(et + 1) * 128, qsl], in_=stg)
                        return run
                    return [(2 * et + 2, piece(et)) for et in range(8)]

                for qt in range(4):
                    attn_block(0, qt,
                               fillers=norm_pieces(qt - 1) if qt >= 1 else ())
                    attn_block(1, qt,
                               fillers=proj_pieces(qt - 1) if qt >= 1 else ())
                for _, f in norm_pieces(3):
                    f()
                for _, f in proj_pieces(3):
                    f()

    nc.finalize()
    return nc


_NC_CACHE = None


def _get_nc():
    global _NC_CACHE
    if _NC_CACHE is None:
        _NC_CACHE = build_nc()
    return _NC_CACHE


def _swz(wT):
    """[C*128, cols] -> DMA-contiguous [128, C*cols] (partition-major)."""
    C = wT.shape[0] // 128
    return np.ascontiguousarray(
        wT.reshape(C, 128, -1).swapaxes(0, 1).reshape(128, -1)).astype(bf16)


def make_in_maps(query, key, value, wq, bq, wk, bk, wv, bv, wo, bo):
    in_maps = []
    for c in range(N_CORES):
        b, hg = divmod(c, HPC)
        sl = slice(hg * W, (hg + 1) * W)
        in_maps.append({
            "xq": np.ascontiguousarray(np.asarray(query[b]).T).astype(bf16),
            "xk": np.ascontiguousarray(np.asarray(key[b]).T).astype(bf16),
            "xv": np.ascontiguousarray(np.asarray(value[b]).T).astype(bf16),
            "wq": _swz(np.asarray(wq)[sl].T),
            "wk": _swz(np.asarray(wk)[sl].T),
            "wv": _swz(np.asarray(wv)[sl].T),
            "wo": _swz(np.asarray(wo)[:, sl].T),
            "bq": np.asarray(bq)[sl].reshape(1, W).astype(bf16),
            "bk": np.asarray(bk)[sl].reshape(1, W).astype(bf16),
            "bv": np.asarray(bv)[sl].reshape(1, W).astype(bf16),
            "ident": np.eye(128, dtype=np.float32).astype(bf16),
        })
    return in_maps


def combine_outputs(outs, bo):
    full = np.zeros((B, T, D), np.float32)
    for c in range(N_CORES):
        b = c // HPC
        full[b] += outs[c].T
    full += np.asarray(bo, np.float32)[None, None, :]
    return full


def kernel(query, key, value, wq, bq, wk, bk, wv, bv, wo, bo):
    nc = _get_nc()
    in_maps = make_in_maps(query, key, value, wq, bq, wk, bk, wv, bv, wo, bo)
    res = run_bass_kernel_spmd(nc, in_maps, list(range(N_CORES)))
    outs = [np.asarray(res.results[c]["out"]) for c in range(N_CORES)]
    return combine_outputs(outs, bo)
